# revision 20
# baseline (speedup 1.0000x reference)
"""Trainium2 Bass kernel for nn_Attention_81037442941065.

Dual-attention module (spatial [b,h,n,n] + channel [b,h,d,d]) with
B=2, N=2048, DIM=1024, 16 heads of d=64.

Sharding: 8 cores = (2 batches) x (4 head-groups of 4 heads).
Each core computes its batch/head-group slice end-to-end and produces a
partial (over head groups) output projection; the host sums the 4 group
partials per batch and adds b_out.

Compute is bf16 end-to-end; fp8e4m3 compresses the x/z input DMA (their
error feeds the attenuated out1/logit paths; |out2| ~ 5x |out1| and the
y path stays bf16).  Engine-balance design (v2):
  * ScalarE runs ONLY the exp stream (the S(j=0) matmul of each
    iteration gates on ScalarE retiring the previous iteration's exps,
    so any copy riding ScalarE stalls the PE at iteration boundaries).
    All psum->SBUF copies (z1T/yhT, final outputs) run on VectorE.
  * Spatial softmax denominators use reciprocal_approx_fast (~5x faster
    than the iterative-divide reciprocal whose 2.7us/call head-of-line
    blocked the in-order DVE queue and stalled psaux release).
  * Tails' broadcast/mul/add run on GpSimd (idle mid-kernel), keeping
    VectorE free for the psum-release copies.
  * Input DMAs are column-split [128,512] and ordered by consumption
    deadline across the three DGE queues, so aux thunks popped into the
    in-order PE queue never wait on distant DMA.
  * Startup memsets (catp, xq ones columns) run on VectorE, off the
    gpsimd DMA queue.
  * The final projection of the last token block is split: the q=0
    (head-pair 0) contraction runs mid-kernel into SBUF partials, so
    the tail after the last AV is only tails + 8 q=1 matmuls + add+DMA.
"""

import os
import sys

for _p in ("/opt/trn_rl_repo", "/opt/pypackages"):
    if _p not in sys.path:
        sys.path.insert(0, _p)

_DBG_RECIP_EXACT = os.environ.get("DBG_RECIP_EXACT", "") == "1"
_DBG_TAILS_VEC = os.environ.get("DBG_TAILS_VEC", "") == "1"
_DBG_MEMSET_GPSIMD = os.environ.get("DBG_MEMSET_GPSIMD", "") == "1"
_DBG_NO_FIN0 = os.environ.get("DBG_NO_FIN0", "") == "1"

import ml_dtypes
import numpy as np
from contextlib import ExitStack

import concourse.bacc as bacc
import concourse.mybir as mybir
import concourse.tile as tile
from concourse.tile import add_dep_helper
from concourse.bass_utils import run_bass_kernel_spmd

F32 = mybir.dt.float32
BF16 = mybir.dt.bfloat16
FP8 = mybir.dt.float8e4
EXP = mybir.ActivationFunctionType.Exp
LOG = mybir.ActivationFunctionType.Ln
COPY = mybir.ActivationFunctionType.Copy

B, N, DIM = 2, 2048, 1024
HEADS, DH = 16, 64
G = 4              # head groups == cores per batch
HG = HEADS // G    # heads per group (4)
CIN = HG * DH      # inner channels per core (256)
NCORES = 8
NCH = N // 128     # 128-token chunks (16)
SCALE = DH ** -0.5            # 1/8
CM_SCALE = SCALE / (N / DH)   # 1/256
XP = DH + 2        # xh_aug head pitch: 64 ch + ones col @64 + pad (66)


def _ride(mm, host, why):
    add_dep_helper(mm.ins, host.ins, sync=False, reason=why)


def _build_program():
    nc = bacc.Bacc(
        "TRN2", target_bir_lowering=False, debug=False, num_devices=NCORES
    )

    # ---- DRAM I/O ----
    xT_d = nc.dram_tensor("xT", [DIM, N], FP8, kind="ExternalInput").ap()
    yT_d = nc.dram_tensor("yT", [DIM, N], BF16, kind="ExternalInput").ap()
    zT_d = nc.dram_tensor("zT", [DIM, N], FP8, kind="ExternalInput").ap()
    wsa1_d = nc.dram_tensor("w_sa1", [DIM, CIN], BF16, kind="ExternalInput").ap()
    wsa2_d = nc.dram_tensor("w_sa2", [DIM, CIN], BF16, kind="ExternalInput").ap()
    wse1_d = nc.dram_tensor("w_se1", [DIM, CIN], BF16, kind="ExternalInput").ap()
    wse2_d = nc.dram_tensor("w_se2", [DIM, CIN], BF16, kind="ExternalInput").ap()
    wout_d = nc.dram_tensor("w_out", [CIN, DIM], BF16, kind="ExternalInput").ap()
    outT_d = nc.dram_tensor("outT", [DIM, N], BF16, kind="ExternalOutput").ap()

    with tile.TileContext(nc) as tc, ExitStack() as ctx:
        ppool = ctx.enter_context(tc.tile_pool(name="persist", bufs=1))
        ipool = ctx.enter_context(tc.tile_pool(name="inputs", bufs=1))
        ptpool = ctx.enter_context(tc.tile_pool(name="pt", bufs=10))
        tpool = ctx.enter_context(tc.tile_pool(name="tails", bufs=1))
        opool = ctx.enter_context(tc.tile_pool(name="oout", bufs=3))
        psS = ctx.enter_context(tc.tile_pool(name="psS", bufs=2, space="PSUM"))
        psAV = ctx.enter_context(tc.tile_pool(name="psAV", bufs=2, space="PSUM"))
        psaux = ctx.enter_context(tc.tile_pool(name="psaux", bufs=2, space="PSUM"))

        # ---- persistent tiles ----
        z1T = [ppool.tile([128, N], BF16, tag=f"z1T{m}", name=f"z1T{m}")
               for m in range(2)]   # head pair m: [2x64 ch, tokens]
        yhT = [ppool.tile([128, N], BF16, tag=f"yhT{m}", name=f"yhT{m}")
               for m in range(2)]
        catp = [ppool.tile([128, N], BF16, tag=f"cat{m}", name=f"cat{m}")
                for m in range(2)]  # head-pair-packed out1+out2
        # xh_aug[i]: [tok128, head, XP]; ch 0..63, ones col @64
        xq = [ppool.tile([128, HG * XP], BF16, tag=f"xq{i}", name=f"xq{i}")
              for i in range(NCH)]
        zq = [ppool.tile([128, HG * DH], BF16, tag=f"zq{i}", name=f"zq{i}")
              for i in range(NCH)]
        secm_sb = [ppool.tile([128, DH], BF16, tag=f"cm{p}", name=f"cm{p}")
                   for p in range(2)]
        rs = [ppool.tile([64, 1], F32, tag=f"rs{h}", name=f"rs{h}")
              for h in range(HG)]
        rcm = [ppool.tile([64, 1], F32, tag=f"rcm{h}", name=f"rcm{h}")
               for h in range(HG)]
        of0 = [ppool.tile([128, 512], BF16, tag=f"of0{d}", name=f"of0{d}")
               for d in range(8)]   # early q=0 final partials, last block

        # ---- input tiles (all [128-dim-chunk, ...]) ----
        xt = [ipool.tile([128, N], FP8, tag=f"xt{k}", name=f"xt{k}")
              for k in range(8)]
        yt = [ipool.tile([128, N], BF16, tag=f"yt{k}", name=f"yt{k}")
              for k in range(8)]
        zt = [ipool.tile([128, N], FP8, tag=f"zt{k}", name=f"zt{k}")
              for k in range(8)]
        wsa1_t = [ipool.tile([128, CIN], BF16, tag=f"wsa1_{k}",
                             name=f"wsa1_{k}") for k in range(8)]
        wsa2_t = [ipool.tile([128, CIN], BF16, tag=f"wsa2_{k}",
                             name=f"wsa2_{k}") for k in range(8)]
        wse1_t = [ipool.tile([128, CIN], BF16, tag=f"wse1_{k}",
                             name=f"wse1_{k}") for k in range(8)]
        wse2_t = [ipool.tile([128, CIN], BF16, tag=f"wse2_{k}",
                             name=f"wse2_{k}") for k in range(8)]
        wp = [ipool.tile([128, DIM], BF16, tag=f"wp{p}", name=f"wp{p}")
              for p in range(2)]

        # ---- startup memsets on VectorE (idle until the first exp) ----
        _mse = nc.gpsimd if _DBG_MEMSET_GPSIMD else nc.vector
        for m in range(2):
            _mse.memset(catp[m][:], 0.0)
        for i in range(NCH):
            _mse.memset(
                xq[i][:].rearrange("p (h c) -> p h c", c=XP)[:, :, DH:DH + 1],
                1.0)

        # ---- input DMAs: wide-row ops, consumption-ordered ----
        # (512B-row col-split DMAs measured ~4x worse byte-efficiency, so
        # only the first-needed column blocks are split off.)
        def dma_cols(q, tiles, dram, c0, c1, ks):
            for k in ks:
                q.dma_start(tiles[k][:, c0:c1], dram[k * 128:(k + 1) * 128,
                                                     c0:c1])

        def dma_full(q, tiles, dram, ks, w=None):
            for k in ks:
                q.dma_start(tiles[k][:], dram[k * 128:(k + 1) * 128, :])

        LO, HI, ALL = range(0, 4), range(4, 8), range(8)
        # gpsimd: zt b0 lo | yt b0 hi | wse1 | zt rest lo | wse2
        dma_cols(nc.gpsimd, zt, zT_d, 0, 512, LO)
        dma_cols(nc.gpsimd, yt, yT_d, 0, 512, HI)
        dma_full(nc.gpsimd, wse1_t, wse1_d, ALL)
        dma_cols(nc.gpsimd, zt, zT_d, 512, 2048, LO)
        dma_full(nc.gpsimd, wse2_t, wse2_d, ALL)
        # sync: wsa1 | zt b0 hi | yt b1 lo | yt b23 hi | xt lo | zt rest hi
        dma_full(nc.sync, wsa1_t, wsa1_d, ALL)
        dma_cols(nc.sync, zt, zT_d, 0, 512, HI)
        dma_cols(nc.sync, yt, yT_d, 512, 1024, LO)
        dma_cols(nc.sync, yt, yT_d, 1024, 2048, HI)
        dma_full(nc.sync, xt, xT_d, LO)
        dma_cols(nc.sync, zt, zT_d, 512, 2048, HI)
        # scalar: wsa2 | yt b0 lo | yt b1 hi | yt b23 lo | xt hi | wp
        dma_full(nc.scalar, wsa2_t, wsa2_d, ALL)
        dma_cols(nc.scalar, yt, yT_d, 0, 512, LO)
        dma_cols(nc.scalar, yt, yT_d, 512, 1024, HI)
        dma_cols(nc.scalar, yt, yT_d, 1024, 2048, LO)
        dma_full(nc.scalar, xt, xT_d, HI)
        for p in range(2):
            nc.scalar.dma_start(wp[p][:], wout_d[p * 128:(p + 1) * 128, :])

        # ================= aux PE unit emitters =================
        def emit_z1T(m, nb):
            ps = psaux.tile([128, 512], F32, tag="aux", name=f"z1p{m}{nb}")
            mm = None
            for k in range(8):
                mm = nc.tensor.matmul(
                    ps[:],
                    lhsT=wsa1_t[k][:, 128 * m:128 * m + 128],
                    rhs=zt[k][:, 512 * nb:512 * nb + 512],
                    start=(k == 0), stop=(k == 7),
                )
            nc.vector.tensor_copy(z1T[m][:, 512 * nb:512 * nb + 512], ps[:])
            return mm

        def emit_yhT(m, nb):
            ps = psaux.tile([128, 512], F32, tag="aux", name=f"yhp{m}{nb}")
            mm = None
            for k in range(8):
                mm = nc.tensor.matmul(
                    ps[:],
                    lhsT=wsa2_t[k][:, 128 * m:128 * m + 128],
                    rhs=yt[k][:, 512 * nb:512 * nb + 512],
                    start=(k == 0), stop=(k == 7),
                )
            nc.vector.tensor_copy(yhT[m][:, 512 * nb:512 * nb + 512], ps[:])
            return mm

        def emit_xh(i):
            # xh token chunk i -> xh_aug[i] (natural layout, M=128)
            ps = psaux.tile([128, 512], F32, tag="aux", name=f"xhp{i}")
            mm = None
            for k in range(8):
                mm = nc.tensor.matmul(
                    ps[:, 0:CIN],
                    lhsT=xt[k][:, 128 * i:128 * i + 128],
                    rhs=wse1_t[k][:],
                    start=(k == 0), stop=(k == 7),
                )
            src = ps[:, 0:CIN].rearrange("p (h c) -> p h c", c=DH)
            dst = xq[i][:].rearrange("p (h c) -> p h c", c=XP)[:, :, 0:DH]
            nc.vector.tensor_copy(dst, src)
            return mm

        def emit_z2(i):
            ps = psaux.tile([128, 512], F32, tag="aux", name=f"z2p{i}")
            mm = None
            for k in range(8):
                mm = nc.tensor.matmul(
                    ps[:, 0:CIN],
                    lhsT=zt[k][:, 128 * i:128 * i + 128],
                    rhs=wse2_t[k][:],
                    start=(k == 0), stop=(k == 7),
                )
            nc.vector.tensor_copy(zq[i][:, 0:CIN], ps[:, 0:CIN])
            return mm

        def emit_channel():
            # channel-attn logits: the 4 heads' [64,64] accumulation groups
            # ride ONE psum group (rows 0-63, col block 64h per head).
            cmp_ = psaux.tile([128, 512], F32, tag="aux", name="cmps")
            start_mm = None
            chain_last = {}
            mm = None
            for i in range(NCH):
                for h in range(HG):
                    mm = nc.tensor.matmul(
                        cmp_[0:64, 64 * h:64 * h + 64],
                        lhsT=xq[i][:, XP * h:XP * h + DH],
                        rhs=zq[i][:, DH * h:DH * h + DH],
                        start=(i == 0 and h == 0),
                        stop=(i == NCH - 1 and h == HG - 1),
                        skip_group_check=True,
                    )
                    if i == 0 and h == 0:
                        start_mm = mm
                    elif i == 0:
                        _ride(mm, start_mm, "rider after group start")
                    if i == NCH - 1 and h < HG - 1:
                        chain_last[h] = mm
            for h in range(HG - 1):
                _ride(mm, chain_last[h], "stop after rider chains")
            for h in range(HG):
                p_, off = h // 2, 64 * (h % 2)
                st = tpool.tile([64, DH], BF16, tag="cmstage",
                                name=f"cmstage{h}")
                nc.scalar.activation(st[:], cmp_[0:64, 64 * h:64 * h + 64],
                                     EXP, scale=CM_SCALE,
                                     accum_out=rs[h][0:64, 0:1])
                nc.vector.reciprocal(rcm[h][0:64, 0:1], rs[h][0:64, 0:1])
                nc.vector.tensor_scalar_mul(st[:], st[:], rcm[h][0:64, 0:1])
                nc.sync.dma_start(secm_sb[p_][off:off + 64, :], st[:])
            return mm

        def emit_out2(h, nb):
            p_, off = h // 2, 64 * (h % 2)
            pso = psaux.tile([128, 512], F32, tag="aux", name=f"pso{h}{nb}")
            mm = nc.tensor.matmul(
                pso[off:off + 64, :],
                lhsT=secm_sb[p_][off:off + 64, :],
                rhs=yhT[p_][off:off + 64, nb * 512:(nb + 1) * 512],
                start=True, stop=True,
            )
            dst = catp[p_][off:off + 64, nb * 512:(nb + 1) * 512]
            nc.vector.tensor_add(dst, pso[off:off + 64, :], dst)
            return mm

        final_psf = {}

        def emit_final(d, nb, q):
            if q == 0:
                final_psf[(d, nb)] = psaux.tile(
                    [128, 512], F32, tag="aux", name=f"psf{d}{nb}")
            psf = final_psf[(d, nb)]
            mm = nc.tensor.matmul(
                psf[:],
                lhsT=wp[q][:, d * 128:(d + 1) * 128],
                rhs=catp[q][:, nb * 512:(nb + 1) * 512],
                start=(q == 0), stop=(q == 1),
            )
            if q == 1:
                ob = opool.tile([128, 512], BF16, tag="ob", name=f"ob{d}{nb}")
                nc.vector.tensor_copy(ob[:], psf[:])
                nc.sync.dma_start(
                    outT_d[d * 128:(d + 1) * 128, nb * 512:(nb + 1) * 512],
                    ob[:],
                )
            return mm

        def emit_fin0(d):
            # last block (nb=3), q=0 contraction early -> SBUF partial
            ps = psaux.tile([128, 512], F32, tag="aux", name=f"f0{d}")
            mm = nc.tensor.matmul(
                ps[:], lhsT=wp[0][:, d * 128:(d + 1) * 128],
                rhs=catp[0][:, 1536:2048], start=True, stop=True,
            )
            nc.vector.tensor_copy(of0[d][:], ps[:])
            return mm

        # ---- labeled aux queue ----
        # Emission order IS a correctness constraint: Tile only sees writes
        # that were already emitted, so consumers force their producers out
        # of the queue with drain_until() before touching the data.
        aux_thunks = []
        aux_done = set()
        cur_anchor = [None]

        def queue(label, fn, *args):
            aux_thunks.append((label, lambda fn=fn, args=args: fn(*args)))

        def pop_one():
            label, thunk = aux_thunks.pop(0)
            mm = thunk()
            aux_done.add(label)
            if cur_anchor[0] is not None and mm is not None:
                add_dep_helper(mm.ins, cur_anchor[0].ins, sync=False,
                               reason="pin aux to drain slot")

        def drain_aux(k):
            for _ in range(k):
                if aux_thunks:
                    pop_one()

        def drain_until(label):
            while label not in aux_done and aux_thunks:
                pop_one()

        # prologue: just enough for S(p_=0, ib=0) to start
        emit_z1T(0, 0)
        aux_done.add(("z1T", 0, 0))
        emit_yhT(0, 0)
        aux_done.add(("yhT", 0, 0))

        queue(("yhT", 0, 1), emit_yhT, 0, 1)
        queue(("yhT", 0, 2), emit_yhT, 0, 2)
        queue(("yhT", 0, 3), emit_yhT, 0, 3)
        for i in range(8):
            queue(("xh", i), emit_xh, i)
        queue(("z1T", 0, 1), emit_z1T, 0, 1)
        queue(("z1T", 0, 2), emit_z1T, 0, 2)
        queue(("z1T", 0, 3), emit_z1T, 0, 3)
        for nb in range(4):
            queue(("yhT", 1, nb), emit_yhT, 1, nb)
        queue(("z1T", 1, 0), emit_z1T, 1, 0)
        queue(("z1T", 1, 1), emit_z1T, 1, 1)
        for i in range(8, NCH):
            queue(("xh", i), emit_xh, i)
        for i in range(NCH):
            queue(("z2", i), emit_z2, i)
        queue(("ch",), emit_channel)
        for nb in range(4):
            for h in range(HG):
                queue(("out2", h, nb), emit_out2, h, nb)
        queue(("z1T", 1, 2), emit_z1T, 1, 2)
        queue(("z1T", 1, 3), emit_z1T, 1, 3)

        def queue_finals(nb):
            drain_until(("out2", HG - 1, nb))
            for d in range(8):
                for q in range(2):
                    queue(("fin", d, nb, q), emit_final, d, nb, q)

        # ================= spatial attention =================
        pt = {}
        tail_q = []   # paced tail thunks from the previous iteration

        def make_tails(p_, ib, avs):
            # Returns a LIST of thunks, run one per j-step of the next
            # iteration.  The iterative-divide reciprocal is the one DVE op
            # expensive enough to head-of-line-block the in-order DVE queue
            # (psaux-release copies sit behind it), so it is sliced into
            # [1,128] pieces paced across j=1..4; everything is complete by
            # ~j6, before queue_finals reads catp at j8.
            icol = ib * 512
            avsbs, dns, rcs, bcs = [], [], [], []

            def t_copy():
                # psum-releasing reads first: rows 0-63 and the denom row
                for hh in range(2):
                    avsb = tpool.tile([64, 512], F32, tag=f"avsb{hh}",
                                      name=f"avsb{p_}{ib}{hh}")
                    nc.vector.tensor_copy(avsb[:], avs[hh][0:64, :])
                    avsbs.append(avsb)
                    dn = tpool.tile([1, 512], F32, tag=f"dn{hh}",
                                    name=f"dn{p_}{ib}{hh}")
                    nc.vector.tensor_copy(dn[:], avs[hh][64:65, :])
                    dns.append(dn)
                    rcs.append(tpool.tile([1, 512], F32, tag=f"rc{hh}",
                                          name=f"rc{p_}{ib}{hh}"))

            def t_recip(q4):
                def f():
                    sl = slice(128 * q4, 128 * q4 + 128)
                    for hh in range(2):
                        nc.vector.reciprocal(rcs[hh][:, sl], dns[hh][:, sl])
                return f

            def t_norm():
                for hh in range(2):
                    bc = tpool.tile([64, 512], F32, tag=f"bc{hh}",
                                    name=f"bc{p_}{ib}{hh}")
                    nc.gpsimd.partition_broadcast(bc[:], rcs[hh][:])
                    bcs.append(bc)
                    if hh == 0:
                        tmp = tpool.tile([64, 512], BF16, tag="tmp0",
                                         name=f"tmp{p_}{ib}0")
                        nc.vector.tensor_mul(tmp[:], avsbs[0][:], bc[:])
                        dst = catp[p_][0:64, icol:icol + 512]
                        nc.vector.tensor_add(dst, tmp[:], dst)
                    else:
                        # catp partitions 64-127: bridge via SBUF->SBUF DMA
                        tmpb = tpool.tile([64, 512], BF16, tag="tmpb",
                                          name=f"tmpb{p_}{ib}")
                        nc.vector.tensor_mul(tmpb[:], avsbs[1][:], bc[:])
                        hstage = tpool.tile([128, 512], BF16, tag="hstg",
                                            name=f"hstg{p_}{ib}")
                        nc.sync.dma_start(hstage[64:128, :], tmpb[:])
                        dst = catp[p_][64:128, icol:icol + 512]
                        nc.vector.tensor_add(dst, hstage[64:128, :], dst)

            return [t_copy, t_recip(0), t_recip(1), t_recip(2), t_recip(3),
                    t_norm]

        for p_ in range(2):
            for ib in range(4):
                icol = ib * 512
                drain_until(("z1T", p_, ib))
                avs = [psAV.tile([128, 512], F32, tag="av",
                                 name=f"av{p_}{ib}{hh}") for hh in range(2)]
                av_next = [0]

                def try_av(limit, p_=p_, avs=avs, av_next=av_next):
                    while av_next[0] < limit and \
                            ("xh", av_next[0]) in aux_done:
                        j = av_next[0]
                        for hh in range(2):
                            h = 2 * p_ + hh
                            nc.tensor.matmul(
                                avs[hh][0:DH + 1, :],
                                lhsT=xq[j][:, XP * h:XP * h + DH + 1],
                                rhs=pt[j][:, 512 * hh:512 * hh + 512],
                                start=(j == 0), stop=(j == NCH - 1),
                            )
                        av_next[0] += 1

                for j in range(NCH):
                    if j % 4 == 0:
                        drain_until(("yhT", p_, j // 4))
                    spt = psS.tile([128, 1024], F32, tag="S",
                                   name=f"S{p_}{ib}{j}")
                    s_anchor = None
                    for hh in range(2):
                        off = 64 * hh
                        s_anchor = nc.tensor.matmul(
                            spt[:, 512 * hh:512 * hh + 512],
                            lhsT=yhT[p_][off:off + 64,
                                         j * 128:(j + 1) * 128],
                            rhs=z1T[p_][off:off + 64, icol:icol + 512],
                            start=True, stop=True,
                        )
                    cur_anchor[0] = s_anchor
                    pt[j] = ptpool.tile([128, 1024], BF16, tag="pt",
                                        name=f"pt{p_}{ib}{j}")
                    nc.scalar.activation(pt[j][:], spt[:], EXP, scale=SCALE)
                    if tail_q:
                        tail_q.pop(0)()
                    if j == 8 and p_ == 1:
                        if ib == 0 and not _DBG_NO_FIN0:
                            # tails(p0,3) emitted at j==0 above; pair-0 out2
                            # for block 3 must also be in before fin0 reads
                            # catp[0] block 3
                            drain_until(("out2", 1, 3))
                            for d in range(8):
                                queue(("fin0", d), emit_fin0, d)
                        elif ib >= 1:
                            queue_finals(ib - 1)
                    try_av(j - 3)
                    drain_aux(2 if (len(aux_thunks) > 48 or
                                    (p_ == 1 and ib == 3)) else 1)
                drain_until(("xh", NCH - 1))
                try_av(NCH)
                tail_q = make_tails(p_, ib, avs)
        cur_anchor[0] = None
        for f in tail_q:
            f()
        drain_aux(len(aux_thunks))
        # tail: q=1 finals of the last block vs the SBUF q=0 partials.
        for d in range(8):
            psf1 = psaux.tile([128, 512], F32, tag="aux", name=f"psf1{d}")
            nc.tensor.matmul(
                psf1[:], lhsT=wp[1][:, d * 128:(d + 1) * 128],
                rhs=catp[1][:, 1536:2048], start=True, stop=True,
            )
            ob = opool.tile([128, 512], BF16, tag="ob", name=f"obt{d}")
            nc.vector.tensor_add(ob[:], psf1[:], of0[d][:])
            nc.sync.dma_start(outT_d[d * 128:(d + 1) * 128, 1536:2048],
                              ob[:])

    nc.compile()
    return nc


_NC_CACHE = {}


def _get_program():
    if "nc" not in _NC_CACHE:
        _NC_CACHE["nc"] = _build_program()
    return _NC_CACHE["nc"]


def _prep_input_maps(x, y, z, w_sa1, w_sa2, w_se1, w_se2, w_out):
    bf16 = lambda a: np.ascontiguousarray(
        np.asarray(a, dtype=np.float32).astype(ml_dtypes.bfloat16))
    fp8 = lambda a: np.ascontiguousarray(
        np.asarray(a, dtype=np.float32).astype(ml_dtypes.float8_e4m3))
    maps = []
    for c in range(NCORES):
        b, g = divmod(c, G)
        sl = slice(g * CIN, (g + 1) * CIN)
        maps.append({
            "xT": fp8(np.asarray(x)[b].T),
            "yT": bf16(np.asarray(y)[b].T),
            "zT": fp8(np.asarray(z)[b].T),
            "w_sa1": bf16(np.asarray(w_sa1)[:, sl]),
            "w_sa2": bf16(np.asarray(w_sa2)[:, sl]),
            "w_se1": bf16(np.asarray(w_se1)[:, sl]),
            "w_se2": bf16(np.asarray(w_se2)[:, sl]),
            "w_out": bf16(np.asarray(w_out)[sl, :]),
        })
    return maps


def run(inputs, trace=False, trace_kwargs=None):
    """Run on hardware; returns (full_output, BassKernelResults)."""
    nc = _get_program()
    in_maps = _prep_input_maps(
        inputs["x"], inputs["y"], inputs["z"],
        inputs["w_sa1"], inputs["w_sa2"], inputs["w_se1"], inputs["w_se2"],
        inputs["w_out"],
    )
    res = run_bass_kernel_spmd(
        nc, in_maps, list(range(NCORES)), trace=trace,
        trace_kwargs=trace_kwargs or {},
    )
    out = np.zeros((B, N, DIM), dtype=np.float32)
    for c in range(NCORES):
        b, _g = divmod(c, G)
        out[b] += np.asarray(res.results[c]["outT"], dtype=np.float32).T
    out += np.asarray(inputs["b_out"], dtype=np.float32)
    return out, res


def kernel(**inputs) -> np.ndarray:
    out, _ = run(inputs, trace=False)
    return out


# revision 22
# speedup vs baseline: 1.3242x; 1.3242x over previous
"""Trainium2 Bass kernel for nn_Attention_81037442941065.

Dual-attention module (spatial [b,h,n,n] + channel [b,h,d,d]) with
B=2, N=2048, DIM=1024, 16 heads of d=64.

Sharding: 8 cores = (2 batches) x (4 head-groups of 4 heads).
Each core computes its batch/head-group slice end-to-end and produces a
partial (over head groups) output projection; the host sums the 4 group
partials per batch and adds b_out.

Compute is bf16 end-to-end; fp8e4m3 compresses the x/z input DMA (their
error feeds the attenuated out1/logit paths; |out2| ~ 5x |out1| and the
y path stays bf16).  Engine-balance design (v2):
  * ScalarE runs ONLY the exp stream (the S(j=0) matmul of each
    iteration gates on ScalarE retiring the previous iteration's exps,
    so any copy riding ScalarE stalls the PE at iteration boundaries).
    All psum->SBUF copies (z1T/yhT, final outputs) run on VectorE.
  * Spatial softmax denominators use reciprocal_approx_fast (~5x faster
    than the iterative-divide reciprocal whose 2.7us/call head-of-line
    blocked the in-order DVE queue and stalled psaux release).
  * Tails' broadcast/mul/add run on GpSimd (idle mid-kernel), keeping
    VectorE free for the psum-release copies.
  * Input DMAs are column-split [128,512] and ordered by consumption
    deadline across the three DGE queues, so aux thunks popped into the
    in-order PE queue never wait on distant DMA.
  * Startup memsets (catp, xq ones columns) run on VectorE, off the
    gpsimd DMA queue.
  * The final projection of the last token block is split: the q=0
    (head-pair 0) contraction runs mid-kernel into SBUF partials, so
    the tail after the last AV is only tails + 8 q=1 matmuls + add+DMA.
"""

import os
import sys

for _p in ("/opt/trn_rl_repo", "/opt/pypackages"):
    if _p not in sys.path:
        sys.path.insert(0, _p)

_DBG_RECIP_EXACT = os.environ.get("DBG_RECIP_EXACT", "") == "1"
_DBG_TAILS_VEC = os.environ.get("DBG_TAILS_VEC", "") == "1"
_DBG_MEMSET_GPSIMD = os.environ.get("DBG_MEMSET_GPSIMD", "") == "1"
_DBG_NO_FIN0 = os.environ.get("DBG_NO_FIN0", "") == "1"

import ml_dtypes
import numpy as np
from contextlib import ExitStack

import concourse.bacc as bacc
import concourse.mybir as mybir
import concourse.tile as tile
from concourse.tile import add_dep_helper
from concourse.bass_utils import run_bass_kernel_spmd

F32 = mybir.dt.float32
BF16 = mybir.dt.bfloat16
FP8 = mybir.dt.float8e4
EXP = mybir.ActivationFunctionType.Exp
LOG = mybir.ActivationFunctionType.Ln
COPY = mybir.ActivationFunctionType.Copy

B, N, DIM = 2, 2048, 1024
HEADS, DH = 16, 64
G = 4              # head groups == cores per batch
HG = HEADS // G    # heads per group (4)
CIN = HG * DH      # inner channels per core (256)
NCORES = 8
NCH = N // 128     # 128-token chunks (16)
SCALE = DH ** -0.5            # 1/8
CM_SCALE = SCALE / (N / DH)   # 1/256
XP = DH + 2        # xh_aug head pitch: 64 ch + ones col @64 + pad (66)


def _ride(mm, host, why):
    add_dep_helper(mm.ins, host.ins, sync=False, reason=why)


def _build_program():
    nc = bacc.Bacc(
        "TRN2", target_bir_lowering=False, debug=False, num_devices=NCORES
    )

    # The act-table chooser maps each function to its "home" set, so a
    # kernel using Exp and Ln ping-pongs between exp_and_others and
    # natural_log_exp_and_others (~2.7us per swap, every iteration).
    # Both functions live in natural_log_exp_and_others; steer Exp there
    # by removing it from the exp_and_others entry (in-place, so the
    # set-id indexing by insertion order is preserved).
    from concourse.hw_specs import get_activation_tables
    tables = get_activation_tables(nc.m.arch)
    if "exp_and_others" in tables and "natural_log_exp_and_others" in tables:
        tables["exp_and_others"].discard(EXP)

    # ---- DRAM I/O ----
    xT_d = nc.dram_tensor("xT", [DIM, N], FP8, kind="ExternalInput").ap()
    yT_d = nc.dram_tensor("yT", [DIM, N], BF16, kind="ExternalInput").ap()
    zT_d = nc.dram_tensor("zT", [DIM, N], FP8, kind="ExternalInput").ap()
    wsa1_d = nc.dram_tensor("w_sa1", [DIM, CIN], BF16, kind="ExternalInput").ap()
    wsa2_d = nc.dram_tensor("w_sa2", [DIM, CIN], BF16, kind="ExternalInput").ap()
    wse1_d = nc.dram_tensor("w_se1", [DIM, CIN], BF16, kind="ExternalInput").ap()
    wse2_d = nc.dram_tensor("w_se2", [DIM, CIN], BF16, kind="ExternalInput").ap()
    wout_d = nc.dram_tensor("w_out", [CIN, DIM], BF16, kind="ExternalInput").ap()
    outT_d = nc.dram_tensor("outT", [DIM, N], BF16, kind="ExternalOutput").ap()

    with tile.TileContext(nc) as tc, ExitStack() as ctx:
        ppool = ctx.enter_context(tc.tile_pool(name="persist", bufs=1))
        ipool = ctx.enter_context(tc.tile_pool(name="inputs", bufs=1))
        ptpool = ctx.enter_context(tc.tile_pool(name="pt", bufs=10))
        tpool = ctx.enter_context(tc.tile_pool(name="tails", bufs=1))
        opool = ctx.enter_context(tc.tile_pool(name="oout", bufs=3))
        psS = ctx.enter_context(tc.tile_pool(name="psS", bufs=2, space="PSUM"))
        psAV = ctx.enter_context(tc.tile_pool(name="psAV", bufs=2, space="PSUM"))
        psaux = ctx.enter_context(tc.tile_pool(name="psaux", bufs=2, space="PSUM"))

        # ---- persistent tiles ----
        z1T = [ppool.tile([128, N], BF16, tag=f"z1T{m}", name=f"z1T{m}")
               for m in range(2)]   # head pair m: [2x64 ch, tokens]
        yhT = [ppool.tile([128, N], BF16, tag=f"yhT{m}", name=f"yhT{m}")
               for m in range(2)]
        catp = [ppool.tile([128, N], BF16, tag=f"cat{m}", name=f"cat{m}")
                for m in range(2)]  # head-pair-packed out1+out2
        # xh_aug[i]: [tok128, head, XP]; ch 0..63, ones col @64
        xq = [ppool.tile([128, HG * XP], BF16, tag=f"xq{i}", name=f"xq{i}")
              for i in range(NCH)]
        zq = [ppool.tile([128, HG * DH], BF16, tag=f"zq{i}", name=f"zq{i}")
              for i in range(NCH)]
        secm_sb = [ppool.tile([128, DH], BF16, tag=f"cm{p}", name=f"cm{p}")
                   for p in range(2)]
        rs = [ppool.tile([64, 1], F32, tag=f"rs{h}", name=f"rs{h}")
              for h in range(HG)]
        rcm = [ppool.tile([64, 1], F32, tag=f"rcm{h}", name=f"rcm{h}")
               for h in range(HG)]
        of0 = [ppool.tile([128, 512], BF16, tag=f"of0{d}", name=f"of0{d}")
               for d in range(8)]   # early q=0 final partials, last block

        # ---- input tiles (all [128-dim-chunk, ...]) ----
        xt = [ipool.tile([128, N], FP8, tag=f"xt{k}", name=f"xt{k}")
              for k in range(8)]
        yt = [ipool.tile([128, N], BF16, tag=f"yt{k}", name=f"yt{k}")
              for k in range(8)]
        zt = [ipool.tile([128, N], FP8, tag=f"zt{k}", name=f"zt{k}")
              for k in range(8)]
        wsa1_t = [ipool.tile([128, CIN], BF16, tag=f"wsa1_{k}",
                             name=f"wsa1_{k}") for k in range(8)]
        wsa2_t = [ipool.tile([128, CIN], BF16, tag=f"wsa2_{k}",
                             name=f"wsa2_{k}") for k in range(8)]
        wse1_t = [ipool.tile([128, CIN], BF16, tag=f"wse1_{k}",
                             name=f"wse1_{k}") for k in range(8)]
        wse2_t = [ipool.tile([128, CIN], BF16, tag=f"wse2_{k}",
                             name=f"wse2_{k}") for k in range(8)]
        wp = [ipool.tile([128, DIM], BF16, tag=f"wp{p}", name=f"wp{p}")
              for p in range(2)]

        # ---- startup memsets on VectorE (idle until the first exp) ----
        _mse = nc.gpsimd if _DBG_MEMSET_GPSIMD else nc.vector
        for m in range(2):
            _mse.memset(catp[m][:], 0.0)
        for i in range(NCH):
            _mse.memset(
                xq[i][:].rearrange("p (h c) -> p h c", c=XP)[:, :, DH:DH + 1],
                1.0)

        # ---- input DMAs: wide-row ops, consumption-ordered ----
        # (512B-row col-split DMAs measured ~4x worse byte-efficiency, so
        # only the first-needed column blocks are split off.)
        def dma_cols(q, tiles, dram, c0, c1, ks):
            for k in ks:
                q.dma_start(tiles[k][:, c0:c1], dram[k * 128:(k + 1) * 128,
                                                     c0:c1])

        def dma_full(q, tiles, dram, ks, w=None):
            for k in ks:
                q.dma_start(tiles[k][:], dram[k * 128:(k + 1) * 128, :])

        LO, HI, ALL = range(0, 4), range(4, 8), range(8)
        # gpsimd: zt b0 lo | yt b0 hi | wse1 | zt rest lo | wse2
        dma_cols(nc.gpsimd, zt, zT_d, 0, 512, LO)
        dma_cols(nc.gpsimd, yt, yT_d, 0, 512, HI)
        dma_full(nc.gpsimd, wse1_t, wse1_d, ALL)
        dma_cols(nc.gpsimd, zt, zT_d, 512, 2048, LO)
        dma_full(nc.gpsimd, wse2_t, wse2_d, ALL)
        # sync: wsa1 | zt b0 hi | yt b1 lo | yt b23 hi | xt lo | zt rest hi
        dma_full(nc.sync, wsa1_t, wsa1_d, ALL)
        dma_cols(nc.sync, zt, zT_d, 0, 512, HI)
        dma_cols(nc.sync, yt, yT_d, 512, 1024, LO)
        dma_cols(nc.sync, yt, yT_d, 1024, 2048, HI)
        dma_full(nc.sync, xt, xT_d, LO)
        dma_cols(nc.sync, zt, zT_d, 512, 2048, HI)
        # scalar: wsa2 | yt b0 lo | yt b1 hi | yt b23 lo | xt hi | wp
        dma_full(nc.scalar, wsa2_t, wsa2_d, ALL)
        dma_cols(nc.scalar, yt, yT_d, 0, 512, LO)
        dma_cols(nc.scalar, yt, yT_d, 512, 1024, HI)
        dma_cols(nc.scalar, yt, yT_d, 1024, 2048, LO)
        dma_full(nc.scalar, xt, xT_d, HI)
        for p in range(2):
            nc.scalar.dma_start(wp[p][:], wout_d[p * 128:(p + 1) * 128, :])

        # ================= aux PE unit emitters =================
        def emit_z1T(m, nb):
            ps = psaux.tile([128, 512], F32, tag="aux", name=f"z1p{m}{nb}")
            mm = None
            for k in range(8):
                mm = nc.tensor.matmul(
                    ps[:],
                    lhsT=wsa1_t[k][:, 128 * m:128 * m + 128],
                    rhs=zt[k][:, 512 * nb:512 * nb + 512],
                    start=(k == 0), stop=(k == 7),
                )
            nc.vector.tensor_copy(z1T[m][:, 512 * nb:512 * nb + 512], ps[:])
            return mm

        def emit_yhT(m, nb):
            ps = psaux.tile([128, 512], F32, tag="aux", name=f"yhp{m}{nb}")
            mm = None
            for k in range(8):
                mm = nc.tensor.matmul(
                    ps[:],
                    lhsT=wsa2_t[k][:, 128 * m:128 * m + 128],
                    rhs=yt[k][:, 512 * nb:512 * nb + 512],
                    start=(k == 0), stop=(k == 7),
                )
            nc.vector.tensor_copy(yhT[m][:, 512 * nb:512 * nb + 512], ps[:])
            return mm

        def emit_xh(i):
            # xh token chunk i -> xh_aug[i] (natural layout, M=128)
            ps = psaux.tile([128, 512], F32, tag="aux", name=f"xhp{i}")
            mm = None
            for k in range(8):
                mm = nc.tensor.matmul(
                    ps[:, 0:CIN],
                    lhsT=xt[k][:, 128 * i:128 * i + 128],
                    rhs=wse1_t[k][:],
                    start=(k == 0), stop=(k == 7),
                )
            src = ps[:, 0:CIN].rearrange("p (h c) -> p h c", c=DH)
            dst = xq[i][:].rearrange("p (h c) -> p h c", c=XP)[:, :, 0:DH]
            nc.vector.tensor_copy(dst, src)
            return mm

        def emit_z2(i):
            ps = psaux.tile([128, 512], F32, tag="aux", name=f"z2p{i}")
            mm = None
            for k in range(8):
                mm = nc.tensor.matmul(
                    ps[:, 0:CIN],
                    lhsT=zt[k][:, 128 * i:128 * i + 128],
                    rhs=wse2_t[k][:],
                    start=(k == 0), stop=(k == 7),
                )
            nc.vector.tensor_copy(zq[i][:, 0:CIN], ps[:, 0:CIN])
            return mm

        def emit_channel():
            # channel-attn logits: the 4 heads' [64,64] accumulation groups
            # ride ONE psum group (rows 0-63, col block 64h per head).
            cmp_ = psaux.tile([128, 512], F32, tag="aux", name="cmps")
            start_mm = None
            chain_last = {}
            mm = None
            for i in range(NCH):
                for h in range(HG):
                    mm = nc.tensor.matmul(
                        cmp_[0:64, 64 * h:64 * h + 64],
                        lhsT=xq[i][:, XP * h:XP * h + DH],
                        rhs=zq[i][:, DH * h:DH * h + DH],
                        start=(i == 0 and h == 0),
                        stop=(i == NCH - 1 and h == HG - 1),
                        skip_group_check=True,
                    )
                    if i == 0 and h == 0:
                        start_mm = mm
                    elif i == 0:
                        _ride(mm, start_mm, "rider after group start")
                    if i == NCH - 1 and h < HG - 1:
                        chain_last[h] = mm
            for h in range(HG - 1):
                _ride(mm, chain_last[h], "stop after rider chains")
            for h in range(HG):
                p_, off = h // 2, 64 * (h % 2)
                st = tpool.tile([64, DH], BF16, tag="cmstage",
                                name=f"cmstage{h}")
                nc.scalar.activation(st[:], cmp_[0:64, 64 * h:64 * h + 64],
                                     EXP, scale=CM_SCALE,
                                     accum_out=rs[h][0:64, 0:1])
                nc.vector.reciprocal(rcm[h][0:64, 0:1], rs[h][0:64, 0:1])
                nc.vector.tensor_scalar_mul(st[:], st[:], rcm[h][0:64, 0:1])
                nc.sync.dma_start(secm_sb[p_][off:off + 64, :], st[:])
            return mm

        def emit_out2(h, nb):
            p_, off = h // 2, 64 * (h % 2)
            pso = psaux.tile([128, 512], F32, tag="aux", name=f"pso{h}{nb}")
            mm = nc.tensor.matmul(
                pso[off:off + 64, :],
                lhsT=secm_sb[p_][off:off + 64, :],
                rhs=yhT[p_][off:off + 64, nb * 512:(nb + 1) * 512],
                start=True, stop=True,
            )
            dst = catp[p_][off:off + 64, nb * 512:(nb + 1) * 512]
            nc.vector.tensor_add(dst, pso[off:off + 64, :], dst)
            return mm

        final_psf = {}

        def emit_final(d, nb, q):
            if q == 0:
                final_psf[(d, nb)] = psaux.tile(
                    [128, 512], F32, tag="aux", name=f"psf{d}{nb}")
            psf = final_psf[(d, nb)]
            mm = nc.tensor.matmul(
                psf[:],
                lhsT=wp[q][:, d * 128:(d + 1) * 128],
                rhs=catp[q][:, nb * 512:(nb + 1) * 512],
                start=(q == 0), stop=(q == 1),
            )
            if q == 1:
                ob = opool.tile([128, 512], BF16, tag="ob", name=f"ob{d}{nb}")
                nc.vector.tensor_copy(ob[:], psf[:])
                nc.sync.dma_start(
                    outT_d[d * 128:(d + 1) * 128, nb * 512:(nb + 1) * 512],
                    ob[:],
                )
            return mm

        def emit_fin0(d):
            # last block (nb=3), q=0 contraction early -> SBUF partial
            ps = psaux.tile([128, 512], F32, tag="aux", name=f"f0{d}")
            mm = nc.tensor.matmul(
                ps[:], lhsT=wp[0][:, d * 128:(d + 1) * 128],
                rhs=catp[0][:, 1536:2048], start=True, stop=True,
            )
            nc.vector.tensor_copy(of0[d][:], ps[:])
            return mm

        # ---- labeled aux queue ----
        # Emission order IS a correctness constraint: Tile only sees writes
        # that were already emitted, so consumers force their producers out
        # of the queue with drain_until() before touching the data.
        aux_thunks = []
        aux_done = set()
        cur_anchor = [None]

        def queue(label, fn, *args):
            aux_thunks.append((label, lambda fn=fn, args=args: fn(*args)))

        def pop_one():
            label, thunk = aux_thunks.pop(0)
            mm = thunk()
            aux_done.add(label)
            if cur_anchor[0] is not None and mm is not None:
                add_dep_helper(mm.ins, cur_anchor[0].ins, sync=False,
                               reason="pin aux to drain slot")

        def drain_aux(k):
            for _ in range(k):
                if aux_thunks:
                    pop_one()

        def drain_until(label):
            while label not in aux_done and aux_thunks:
                pop_one()

        # prologue: just enough for S(p_=0, ib=0) to start
        emit_z1T(0, 0)
        aux_done.add(("z1T", 0, 0))
        emit_yhT(0, 0)
        aux_done.add(("yhT", 0, 0))

        queue(("yhT", 0, 1), emit_yhT, 0, 1)
        queue(("yhT", 0, 2), emit_yhT, 0, 2)
        queue(("yhT", 0, 3), emit_yhT, 0, 3)
        for i in range(8):
            queue(("xh", i), emit_xh, i)
        queue(("z1T", 0, 1), emit_z1T, 0, 1)
        queue(("z1T", 0, 2), emit_z1T, 0, 2)
        queue(("z1T", 0, 3), emit_z1T, 0, 3)
        for nb in range(4):
            queue(("yhT", 1, nb), emit_yhT, 1, nb)
        queue(("z1T", 1, 0), emit_z1T, 1, 0)
        queue(("z1T", 1, 1), emit_z1T, 1, 1)
        for i in range(8, NCH):
            queue(("xh", i), emit_xh, i)
        for i in range(NCH):
            queue(("z2", i), emit_z2, i)
        queue(("ch",), emit_channel)
        for nb in range(4):
            for h in range(HG):
                queue(("out2", h, nb), emit_out2, h, nb)
        queue(("z1T", 1, 2), emit_z1T, 1, 2)
        queue(("z1T", 1, 3), emit_z1T, 1, 3)

        def queue_finals(nb):
            drain_until(("out2", HG - 1, nb))
            for d in range(8):
                for q in range(2):
                    queue(("fin", d, nb, q), emit_final, d, nb, q)

        # ================= spatial attention =================
        pt = {}
        tail_q = []   # paced tail thunks from the previous iteration

        def make_tails(p_, ib, avs):
            # Returns a LIST of thunks, run one per j-step of the next
            # iteration.  The softmax-denominator reciprocal runs on
            # ScalarE as exp(-ln(d)) (both functions in the natural_log
            # table set, see the act-table steer in _build_program): the
            # iterative-divide DVE reciprocal (~2.7us for [1,512], ~1us
            # minimum per op) would head-of-line-block the in-order DVE
            # queue that all psum-release copies ride on.
            icol = ib * 512
            avsbs, lns, rcs = [], [], []

            def t_copy():
                # psum-releasing reads first: avsb copy (rows 0-63, DVE)
                # and ln of the denominator row (ScalarE reads PSUM)
                for hh in range(2):
                    ln = tpool.tile([1, 512], F32, tag=f"ln{hh}",
                                    name=f"ln{p_}{ib}{hh}")
                    nc.scalar.activation(ln[:], avs[hh][64:65, :], LOG)
                    lns.append(ln)
                    avsb = tpool.tile([64, 512], F32, tag=f"avsb{hh}",
                                      name=f"avsb{p_}{ib}{hh}")
                    nc.vector.tensor_copy(avsb[:], avs[hh][0:64, :])
                    avsbs.append(avsb)

            def t_rc():
                for hh in range(2):
                    rc = tpool.tile([1, 512], F32, tag=f"rc{hh}",
                                    name=f"rc{p_}{ib}{hh}")
                    nc.scalar.activation(rc[:], lns[hh][:], EXP, scale=-1.0)
                    rcs.append(rc)

            def t_norm():
                for hh in range(2):
                    bc = tpool.tile([64, 512], F32, tag=f"bc{hh}",
                                    name=f"bc{p_}{ib}{hh}")
                    nc.gpsimd.partition_broadcast(bc[:], rcs[hh][:])
                    if hh == 0:
                        tmp = tpool.tile([64, 512], BF16, tag="tmp0",
                                         name=f"tmp{p_}{ib}0")
                        nc.vector.tensor_mul(tmp[:], avsbs[0][:], bc[:])
                        dst = catp[p_][0:64, icol:icol + 512]
                        nc.vector.tensor_add(dst, tmp[:], dst)
                    else:
                        # catp partitions 64-127: bridge via SBUF->SBUF DMA
                        tmpb = tpool.tile([64, 512], BF16, tag="tmpb",
                                          name=f"tmpb{p_}{ib}")
                        nc.vector.tensor_mul(tmpb[:], avsbs[1][:], bc[:])
                        hstage = tpool.tile([128, 512], BF16, tag="hstg",
                                            name=f"hstg{p_}{ib}")
                        nc.sync.dma_start(hstage[64:128, :], tmpb[:])
                        dst = catp[p_][64:128, icol:icol + 512]
                        nc.vector.tensor_add(dst, hstage[64:128, :], dst)

            return [t_copy, t_rc, t_norm]

        for p_ in range(2):
            for ib in range(4):
                icol = ib * 512
                drain_until(("z1T", p_, ib))
                avs = [psAV.tile([128, 512], F32, tag="av",
                                 name=f"av{p_}{ib}{hh}") for hh in range(2)]
                av_next = [0]

                def try_av(limit, p_=p_, avs=avs, av_next=av_next):
                    while av_next[0] < limit and \
                            ("xh", av_next[0]) in aux_done:
                        j = av_next[0]
                        for hh in range(2):
                            h = 2 * p_ + hh
                            nc.tensor.matmul(
                                avs[hh][0:DH + 1, :],
                                lhsT=xq[j][:, XP * h:XP * h + DH + 1],
                                rhs=pt[j][:, 512 * hh:512 * hh + 512],
                                start=(j == 0), stop=(j == NCH - 1),
                            )
                        av_next[0] += 1

                for j in range(NCH):
                    if j % 4 == 0:
                        drain_until(("yhT", p_, j // 4))
                    spt = psS.tile([128, 1024], F32, tag="S",
                                   name=f"S{p_}{ib}{j}")
                    s_anchor = None
                    for hh in range(2):
                        off = 64 * hh
                        s_anchor = nc.tensor.matmul(
                            spt[:, 512 * hh:512 * hh + 512],
                            lhsT=yhT[p_][off:off + 64,
                                         j * 128:(j + 1) * 128],
                            rhs=z1T[p_][off:off + 64, icol:icol + 512],
                            start=True, stop=True,
                        )
                    cur_anchor[0] = s_anchor
                    pt[j] = ptpool.tile([128, 1024], BF16, tag="pt",
                                        name=f"pt{p_}{ib}{j}")
                    nc.scalar.activation(pt[j][:], spt[:], EXP, scale=SCALE)
                    if tail_q:
                        tail_q.pop(0)()
                    if j == 8 and p_ == 1:
                        if ib == 0 and not _DBG_NO_FIN0:
                            # tails(p0,3) emitted at j==0 above; pair-0 out2
                            # for block 3 must also be in before fin0 reads
                            # catp[0] block 3
                            drain_until(("out2", 1, 3))
                            for d in range(8):
                                queue(("fin0", d), emit_fin0, d)
                        elif ib >= 1:
                            queue_finals(ib - 1)
                    try_av(j - 3)
                    drain_aux(2 if (len(aux_thunks) > 48 or
                                    (p_ == 1 and ib == 3)) else 1)
                drain_until(("xh", NCH - 1))
                try_av(NCH)
                tail_q = make_tails(p_, ib, avs)
        cur_anchor[0] = None
        for f in tail_q:
            f()
        drain_aux(len(aux_thunks))
        # tail: q=1 finals of the last block vs the SBUF q=0 partials.
        for d in range(8):
            psf1 = psaux.tile([128, 512], F32, tag="aux", name=f"psf1{d}")
            nc.tensor.matmul(
                psf1[:], lhsT=wp[1][:, d * 128:(d + 1) * 128],
                rhs=catp[1][:, 1536:2048], start=True, stop=True,
            )
            ob = opool.tile([128, 512], BF16, tag="ob", name=f"obt{d}")
            nc.vector.tensor_add(ob[:], psf1[:], of0[d][:])
            nc.sync.dma_start(outT_d[d * 128:(d + 1) * 128, 1536:2048],
                              ob[:])

    nc.compile()
    return nc


_NC_CACHE = {}


def _get_program():
    if "nc" not in _NC_CACHE:
        _NC_CACHE["nc"] = _build_program()
    return _NC_CACHE["nc"]


def _prep_input_maps(x, y, z, w_sa1, w_sa2, w_se1, w_se2, w_out):
    bf16 = lambda a: np.ascontiguousarray(
        np.asarray(a, dtype=np.float32).astype(ml_dtypes.bfloat16))
    fp8 = lambda a: np.ascontiguousarray(
        np.asarray(a, dtype=np.float32).astype(ml_dtypes.float8_e4m3))
    maps = []
    for c in range(NCORES):
        b, g = divmod(c, G)
        sl = slice(g * CIN, (g + 1) * CIN)
        maps.append({
            "xT": fp8(np.asarray(x)[b].T),
            "yT": bf16(np.asarray(y)[b].T),
            "zT": fp8(np.asarray(z)[b].T),
            "w_sa1": bf16(np.asarray(w_sa1)[:, sl]),
            "w_sa2": bf16(np.asarray(w_sa2)[:, sl]),
            "w_se1": bf16(np.asarray(w_se1)[:, sl]),
            "w_se2": bf16(np.asarray(w_se2)[:, sl]),
            "w_out": bf16(np.asarray(w_out)[sl, :]),
        })
    return maps


def run(inputs, trace=False, trace_kwargs=None):
    """Run on hardware; returns (full_output, BassKernelResults)."""
    nc = _get_program()
    in_maps = _prep_input_maps(
        inputs["x"], inputs["y"], inputs["z"],
        inputs["w_sa1"], inputs["w_sa2"], inputs["w_se1"], inputs["w_se2"],
        inputs["w_out"],
    )
    res = run_bass_kernel_spmd(
        nc, in_maps, list(range(NCORES)), trace=trace,
        trace_kwargs=trace_kwargs or {},
    )
    out = np.zeros((B, N, DIM), dtype=np.float32)
    for c in range(NCORES):
        b, _g = divmod(c, G)
        out[b] += np.asarray(res.results[c]["outT"], dtype=np.float32).T
    out += np.asarray(inputs["b_out"], dtype=np.float32)
    return out, res


def kernel(**inputs) -> np.ndarray:
    out, _ = run(inputs, trace=False)
    return out
